# revision 12
# baseline (speedup 1.0000x reference)
"""ViT attention block (qkv -> 12-head softmax attn + depthwise 3x3 conv on V -> proj)
on 8 TRN2 NeuronCores, data-parallel over batch.

Shapes (hardcoded): x [256, 197, 768], W_qkv [768, 2304], W_proj [768, 768],
dwc_w [768, 1, 3, 3]. 8 cores x 32 images each. Host pre-transposes x to
x^T [32, 768, 197] per core (bf16) and posts-transposes the per-core outputs
y^T [32, 768, 197] (f32) back.

Kernel layout strategy (per image):
  - qkv^T = W_qkv^T-chunks @ x^T: psum [128, 2img, 197] per output chunk m (18),
    accumulated over 6 k-chunks; bf16 matmuls at N=394 (image pairs).
  - scores^T[j, i] per head via lhsT=k^T[64, j-chunk], rhs=q^T[64, 197]
    (head pairs auto row-pack at partitions 0/64); exp on ACT (scale=1/8,
    no max-subtraction -- logits are O(1)); output bf16.
  - v transposed to token-major via PE transpose; ones column appended so
    attn@v yields both the unnormalized output and the softmax denominator.
  - attn@v: lhsT=exp^T chunks, rhs=[v|1]; normalize with DVE reciprocal+mult.
  - depthwise conv runs on v^T (channel-major) as 9 fused scale-MAC DVE ops,
    batched over a group of G images.
  - O^T via PE transpose, conv added during psum->sbuf assembly; proj with
    bias as an extra K=1 matmul row; result DMA'd straight from PSUM.
"""

import os
import sys
import types

sys.path.insert(0, "/opt/trn_rl_repo")

import numpy as np
import ml_dtypes

import concourse.bass as bass
import concourse.mybir as mybir
import concourse.tile as tile
from concourse import bacc
from concourse.bass_utils import run_bass_kernel_spmd
from concourse.masks import make_identity

BF16 = mybir.dt.bfloat16
F32 = mybir.dt.float32
AF = mybir.ActivationFunctionType
ALU = mybir.AluOpType

NCORES = 8
B, N, C, H = 256, 197, 768, 12
D = C // H            # 64
NIMG = B // NCORES    # 32 images per core
G = 4                 # images per conv group
NGRP = NIMG // G      # 8 groups
NPAIR = 2             # pairs per group (G/2)
KC = C // 128         # 6 k-chunks
MQKV = 3 * C // 128   # 18 output chunks of qkv
GRID = 14

_cache = {}


def _ensure_profile_hook():
    """bass_utils' axon trace path needs antenv.axon_hooks, which this image
    lacks; inject an equivalent built from trn_agent_boot's ctypes shim."""
    if "antenv.axon_hooks" in sys.modules:
        return
    try:
        from trn_agent_boot.trn_boot import _ntff_profile_via_ctypes

        mod = types.ModuleType("antenv.axon_hooks")
        hook = _ntff_profile_via_ctypes("/opt/axon/libaxon_pjrt.so")
        mod.get_axon_ntff_profile_hook = lambda: hook
        mod.set_axon_ntff_profile_hook = lambda h: None
        sys.modules["antenv.axon_hooks"] = mod
    except Exception:
        pass


def build_program(nimg=NIMG):
    ngrp = nimg // G
    nc = bacc.Bacc("TRN2", target_bir_lowering=False, debug=False,
                   num_devices=NCORES)

    xT_d = nc.dram_tensor("xT", [nimg // 2, 128, KC, 2, N], BF16, kind="ExternalInput").ap()
    wqkv_d = nc.dram_tensor("wqkv", [128, KC, 3 * C], BF16, kind="ExternalInput").ap()
    wproj_d = nc.dram_tensor("wproj", [128, KC, C], BF16, kind="ExternalInput").ap()
    bqkv_d = nc.dram_tensor("bqkv", [128, MQKV], F32, kind="ExternalInput").ap()
    bprojrow_d = nc.dram_tensor("bprojrow", [1, C], BF16, kind="ExternalInput").ap()
    wdwc_d = nc.dram_tensor("wdwc", [128, KC, 9], F32, kind="ExternalInput").ap()
    wdwcn_d = nc.dram_tensor("wdwcn", [128, KC, 9], F32, kind="ExternalInput").ap()
    bdwc_d = nc.dram_tensor("bdwc", [128, KC], F32, kind="ExternalInput").ap()
    yT_d = nc.dram_tensor("yT", [nimg // 2, 128, KC, 2, N], F32, kind="ExternalOutput").ap()

    from contextlib import ExitStack
    with tile.TileContext(nc) as tc, ExitStack() as ctx:
        if True:
            P = lambda *a, **k: ctx.enter_context(tc.tile_pool(*a, **k))
            const = P(name="const", bufs=1)
            xin = P(name="xin", bufs=3)
            qkp = P(name="qk", bufs=2)
            vtp = P(name="vtp", bufs=2)
            convp = P(name="convp", bufs=2)
            gscr = P(name="gscr", bufs=2)
            expp = P(name="expp", bufs=3)
            vtok = P(name="vtok", bufs=2)
            qpadp = P(name="qpadp", bufs=2)
            obuf = P(name="obuf", bufs=G + 1)
            otp = P(name="otp", bufs=2)
            smallp = P(name="smallp", bufs=4)
            ystp = P(name="ystp", bufs=2)
            ps1 = P(name="ps1", bufs=6, space="PSUM")
            ps2 = P(name="ps2", bufs=2, space="PSUM")

            # ---- constants ----
            wqkv_sb = const.tile([128, KC, 3 * C], BF16)
            nc.sync.dma_start(wqkv_sb, wqkv_d)
            wproj_sb = const.tile([128, KC, C], BF16)
            nc.sync.dma_start(wproj_sb, wproj_d)
            bqkv_sb = const.tile([128, MQKV], F32)
            nc.sync.dma_start(bqkv_sb, bqkv_d)
            bprojrow_sb = const.tile([1, C], BF16)
            nc.sync.dma_start(bprojrow_sb, bprojrow_d)
            wdwc_sb = const.tile([128, KC, 9], F32)
            nc.sync.dma_start(wdwc_sb, wdwc_d)
            wdwcn_sb = const.tile([128, KC, 9], F32)
            nc.sync.dma_start(wdwcn_sb, wdwcn_d)
            bdwc_sb = const.tile([128, KC], F32)
            nc.sync.dma_start(bdwc_sb, bdwc_d)
            ident = const.tile([128, 128], BF16)
            make_identity(nc, ident)
            ones_row = const.tile([1, 2, N], BF16)
            nc.vector.memset(ones_row, 1.0)

            for grp in range(ngrp):
                vt_g = vtp.tile([128, KC, G, N], BF16, name="vt_g")
                o_tiles = []

                for p2 in range(NPAIR):
                    img0 = grp * G + p2 * 2
                    xt_p = xin.tile([128, KC, 2, N], BF16, name="xt_p")
                    nc.sync.dma_start(xt_p, xT_d[img0 // 2])
                    qkT_p = qkp.tile([128, 2 * KC, 2, N], BF16, name="qkT_p")

                    # ---- qkv projection (transposed output) ----
                    for m in range(MQKV):
                        ps_qkv = ps1.tile([128, 2, N], F32, name="ps_qkv", tag="ps1")
                        for kc in range(KC):
                            nc.tensor.matmul(
                                ps_qkv,
                                lhsT=wqkv_sb[:, kc, m * 128:(m + 1) * 128],
                                rhs=xt_p[:, kc],
                                start=(kc == 0), stop=(kc == KC - 1),
                            )
                        if m < 2 * KC:
                            nc.scalar.activation(
                                qkT_p[:, m], ps_qkv, AF.Identity,
                                bias=bqkv_sb[:, m:m + 1],
                            )
                        else:
                            nc.scalar.activation(
                                vt_g[:, m - 2 * KC, 2 * p2:2 * p2 + 2], ps_qkv,
                                AF.Identity, bias=bqkv_sb[:, m:m + 1],
                            )

                    # ---- attention per image ----
                    for par in range(2):
                        gi = 2 * p2 + par  # index in group

                        # zero-padded q for odd heads: rows 0:64 zero so a
                        # full-K=128 matmul only sees the odd head's 64 dims
                        # (operands at base_partition 64 crash this stack).
                        qpad = qpadp.tile([128, KC, N], BF16, name="qpad")
                        nc.gpsimd.memset(qpad[0:64], 0.0)
                        for mch in range(KC):
                            nc.gpsimd.tensor_copy(
                                qpad[64:128, mch], qkT_p[64:128, mch, par])

                        # v -> token-major (two token chunks), with ones column
                        v0 = vtok.tile([128, H, D + 1], BF16, name="v0")
                        v1 = vtok.tile([128, H, D + 1], BF16, name="v1")
                        ps_v0 = ps2.tile([128, KC, 128], BF16, name="ps_v0", tag="ps2")
                        for cb in range(KC):
                            nc.tensor.transpose(ps_v0[:, cb], vt_g[:, cb, gi, 0:128], ident)
                        nc.vector.tensor_copy(
                            v0[:, :, 0:D],
                            ps_v0.rearrange("p a (b d) -> p (a b) d", d=D),
                        )
                        nc.vector.memset(v0[:, :, D:D + 1], 1.0)
                        ps_v1 = ps2.tile([128, KC, 128], BF16, name="ps_v1", tag="ps2")
                        for cb in range(KC):
                            nc.tensor.transpose(ps_v1[0:69, cb], vt_g[:, cb, gi, 128:N], ident)
                        nc.vector.tensor_copy(
                            v1[0:69, :, 0:D],
                            ps_v1[0:69].rearrange("p a (b d) -> p (a b) d", d=D),
                        )
                        nc.vector.memset(v1[0:69, :, D:D + 1], 1.0)

                        O0 = obuf.tile([128, H, D], BF16, name="O0")
                        O1 = obuf.tile([128, H, D], BF16, name="O1")
                        o_tiles.append((O0, O1))

                        for hg in range(2):  # two 6-head groups
                            ps_o0 = ps1.tile([128, 6, D + 1], F32, name="ps_o0", tag="ps1")
                            ps_o1 = ps1.tile([128, 6, D + 1], F32, name="ps_o1", tag="ps1")
                            for hp in range(3):  # head pairs within group
                                ps_s0 = ps1.tile([128, 2, N], F32, name="ps_s0", tag="ps1")
                                ps_s1 = ps1.tile([128, 2, N], F32, name="ps_s1", tag="ps1")
                                mch = 3 * hg + hp  # m-chunk (pair of heads)
                                qTe = qkT_p[0:64, mch, par]
                                kTe = qkT_p[0:64, KC + mch, par]
                                nc.tensor.matmul(
                                    ps_s0[:, 0], lhsT=kTe[:, 0:128], rhs=qTe,
                                    start=True, stop=True)
                                nc.tensor.matmul(
                                    ps_s1[0:69, 0], lhsT=kTe[:, 128:N], rhs=qTe,
                                    start=True, stop=True)
                                kTf = qkT_p[:, KC + mch, par]
                                qTo = qpad[:, mch]
                                nc.tensor.matmul(
                                    ps_s0[:, 1], lhsT=kTf[:, 0:128], rhs=qTo,
                                    start=True, stop=True)
                                nc.tensor.matmul(
                                    ps_s1[0:69, 1], lhsT=kTf[:, 128:N], rhs=qTo,
                                    start=True, stop=True)
                                e0 = expp.tile([128, 2, N], BF16, name="e0")
                                e1 = expp.tile([128, 2, N], BF16, name="e1")
                                nc.scalar.activation(e0, ps_s0, AF.Exp, scale=D ** -0.5)
                                nc.scalar.activation(e1[0:69], ps_s1[0:69], AF.Exp,
                                                     scale=D ** -0.5)
                                for sub in range(2):
                                    h = 2 * mch + sub
                                    hs = h % 6
                                    nc.tensor.matmul(
                                        ps_o0[:, hs], lhsT=e0[:, sub, 0:128],
                                        rhs=v0[:, h], start=True, stop=False)
                                    nc.tensor.matmul(
                                        ps_o0[:, hs], lhsT=e1[0:69, sub, 0:128],
                                        rhs=v1[0:69, h], start=False, stop=True)
                                    nc.tensor.matmul(
                                        ps_o1[0:69, hs], lhsT=e0[:, sub, 128:N],
                                        rhs=v0[:, h], start=True, stop=False)
                                    nc.tensor.matmul(
                                        ps_o1[0:69, hs], lhsT=e1[0:69, sub, 128:N],
                                        rhs=v1[0:69, h], start=False, stop=True)
                            # normalize the 6-head group
                            rd0 = smallp.tile([128, 6, 1], F32, name="rd0")
                            nc.vector.reciprocal(rd0, ps_o0[:, :, D:D + 1])
                            nc.vector.tensor_tensor(
                                O0[:, 6 * hg:6 * hg + 6], ps_o0[:, :, 0:D],
                                rd0.to_broadcast([128, 6, D]), ALU.mult)
                            rd1 = smallp.tile([128, 6, 1], F32, name="rd1")
                            nc.vector.reciprocal(rd1[0:69], ps_o1[0:69, :, D:D + 1])
                            nc.vector.tensor_tensor(
                                O1[0:69, 6 * hg:6 * hg + 6], ps_o1[0:69, :, 0:D],
                                rd1[0:69].to_broadcast([69, 6, D]), ALU.mult)

                # ---- depthwise conv over the group (channel-major v^T) ----
                # Flat-shifted taps over the 196-pixel space (3D APs only:
                # walrus rejects 4D). x-edge wraps are fixed up by subtracting
                # the wrongly-added strided column. Two accumulators so the
                # tap chains run on DVE and GPSIMD in parallel.
                acc = convp.tile([128, KC, G, GRID * GRID], BF16, name="acc")
                accb = convp.tile([128, KC, G, GRID * GRID], BF16, name="accb")
                NPX = GRID * GRID
                DVE_TAPS = {(-1, -1), (-1, 0), (-1, 1), (0, -1), (0, 1)}
                for cb in range(KC):
                    vflat = vt_g[:, cb]          # [128, G, 197] tokens
                    vpxv = vt_g[:, cb, :, 1:N].rearrange(
                        "p g (y x) -> p g y x", x=GRID)
                    af = acc[:, cb]
                    bf_ = accb[:, cb]
                    av = acc[:, cb].rearrange("p g (y x) -> p g y x", x=GRID)
                    bv = accb[:, cb].rearrange("p g (y x) -> p g y x", x=GRID)
                    # init: center tap (+bias) on DVE acc; memset on gpsimd acc
                    nc.vector.tensor_scalar(
                        af, vflat[:, :, 1:N], wdwc_sb[:, cb, 4:5],
                        bdwc_sb[:, cb:cb + 1], ALU.mult, ALU.add)
                    nc.gpsimd.memset(bf_, 0.0)
                    for dy in (-1, 0, 1):
                        for dx in (-1, 0, 1):
                            if dy == 0 and dx == 0:
                                continue
                            on_dve = (dy, dx) in DVE_TAPS
                            s = GRID * dy + dx
                            tap = 3 * (dy + 1) + (dx + 1)
                            w = wdwc_sb[:, cb, tap:tap + 1]
                            wn = wdwcn_sb[:, cb, tap:tap + 1]
                            a0, b0 = max(0, -s), NPX - max(0, s)
                            if dx == 0:
                                fix = None
                            else:
                                xo = GRID - 1 if dx == 1 else 0
                                ys = [y for y in range(GRID)
                                      if a0 <= GRID * y + xo < b0]
                                y0f, y1f = ys[0], ys[-1] + 1
                                yi0 = y0f + dy + (1 if dx == 1 else -1)
                                xi = 0 if dx == 1 else GRID - 1
                                fix = (xo, y0f, y1f, yi0, xi)
                            if on_dve:
                                nc.vector.scalar_tensor_tensor(
                                    af[:, :, a0:b0],
                                    vflat[:, :, 1 + a0 + s:1 + b0 + s],
                                    w, af[:, :, a0:b0], ALU.mult, ALU.add)
                                if fix is not None:
                                    xo, y0f, y1f, yi0, xi = fix
                                    nc.vector.scalar_tensor_tensor(
                                        av[:, :, y0f:y1f, xo],
                                        vpxv[:, :, yi0:yi0 + (y1f - y0f), xi],
                                        wn, av[:, :, y0f:y1f, xo],
                                        ALU.mult, ALU.add)
                            else:
                                # gpsimd has no fused MAC: mul to scratch, add
                                tmpg = gscr.tile([128, G, NPX], BF16, name="tmpg")
                                nc.gpsimd.tensor_scalar_mul(
                                    tmpg[:, :, 0:b0 - a0],
                                    vflat[:, :, 1 + a0 + s:1 + b0 + s], w)
                                nc.gpsimd.tensor_tensor(
                                    bf_[:, :, a0:b0], bf_[:, :, a0:b0],
                                    tmpg[:, :, 0:b0 - a0], ALU.add)
                                if fix is not None:
                                    xo, y0f, y1f, yi0, xi = fix
                                    tmpc = gscr.tile([128, G, GRID], BF16,
                                                     name="tmpc")
                                    nc.gpsimd.tensor_scalar_mul(
                                        tmpc[:, :, 0:y1f - y0f],
                                        vpxv[:, :, yi0:yi0 + (y1f - y0f), xi], w)
                                    nc.gpsimd.tensor_tensor(
                                        bv[:, :, y0f:y1f, xo],
                                        bv[:, :, y0f:y1f, xo],
                                        tmpc[:, :, 0:y1f - y0f], ALU.subtract)
                    # merge gpsimd accumulator into the DVE one
                    nc.vector.tensor_tensor(af, af, bf_, ALU.add)
                accf = acc

                # ---- O^T assembly + proj per pair ----
                for p2 in range(NPAIR):
                    img0 = grp * G + p2 * 2
                    OT_p = otp.tile([128, KC, 2, N], BF16, name="OT_p")
                    for par in range(2):
                        gi = 2 * p2 + par
                        O0, O1 = o_tiles[gi]
                        O0v = O0.rearrange("p (c x) d -> p c (x d)", x=2)
                        O1v = O1.rearrange("p (c x) d -> p c (x d)", x=2)
                        ps_t0 = ps2.tile([128, KC, 128], BF16, name="ps_t0", tag="ps2")
                        for cb in range(KC):
                            nc.tensor.transpose(ps_t0[:, cb], O0v[:, cb], ident)
                        nc.vector.tensor_copy(OT_p[:, :, par, 0:1], ps_t0[:, :, 0:1])
                        nc.vector.tensor_tensor(
                            OT_p[:, :, par, 1:128], ps_t0[:, :, 1:128],
                            accf[:, :, gi, 0:127], ALU.add)
                        ps_t1 = ps2.tile([128, KC, 128], BF16, name="ps_t1", tag="ps2")
                        for cb in range(KC):
                            nc.tensor.transpose(ps_t1[:, cb, 0:69], O1v[0:69, cb], ident[0:69, 0:69])
                        nc.vector.tensor_tensor(
                            OT_p[:, :, par, 128:N], ps_t1[:, :, 0:69],
                            accf[:, :, gi, 127:196], ALU.add)

                    yst = ystp.tile([128, KC, 2, N], F32, name="yst")
                    for m in range(KC):
                        ps_y = ps1.tile([128, 2, N], F32, name="ps_y", tag="ps1")
                        for kc in range(KC):
                            nc.tensor.matmul(
                                ps_y, lhsT=wproj_sb[:, kc, m * 128:(m + 1) * 128],
                                rhs=OT_p[:, kc], start=(kc == 0), stop=False)
                        nc.tensor.matmul(
                            ps_y, lhsT=bprojrow_sb[:, m * 128:(m + 1) * 128],
                            rhs=ones_row, start=False, stop=True)
                        nc.scalar.activation(yst[:, m], ps_y, AF.Copy)
                    nc.sync.dma_start(yT_d[img0 // 2], yst)

    nc.compile()
    return nc


def _get_program():
    if "nc" not in _cache:
        _cache["nc"] = build_program()
    return _cache["nc"]


def kernel(x, W_qkv, b_qkv, W_proj, b_proj, dwc_w, dwc_b, _trace=False):
    _ensure_profile_hook()
    nc = _get_program()

    x = np.asarray(x, dtype=np.float32)
    bf = ml_dtypes.bfloat16

    # host-side prep (not on the measured HW path)
    xT = np.ascontiguousarray(
        x.reshape(NCORES, NIMG // 2, 2, N, KC, 128).transpose(0, 1, 5, 4, 2, 3)
    ).astype(bf)
    wqkv = np.ascontiguousarray(
        np.asarray(W_qkv, np.float32).reshape(KC, 128, 3 * C).transpose(1, 0, 2)
    ).astype(bf)
    wproj = np.ascontiguousarray(
        np.asarray(W_proj, np.float32).reshape(KC, 128, C).transpose(1, 0, 2)
    ).astype(bf)
    bqkv = np.ascontiguousarray(
        np.asarray(b_qkv, np.float32).reshape(MQKV, 128).T)
    bprojrow = np.asarray(b_proj, np.float32).reshape(1, C).astype(bf)
    wdwc = np.ascontiguousarray(
        np.asarray(dwc_w, np.float32).reshape(C, 9).reshape(KC, 128, 9)
        .transpose(1, 0, 2))
    wdwcn = np.ascontiguousarray(-wdwc)
    bdwc = np.ascontiguousarray(np.asarray(dwc_b, np.float32).reshape(KC, 128).T)

    in_maps = []
    for c in range(NCORES):
        in_maps.append({
            "xT": xT[c],
            "wqkv": wqkv,
            "wproj": wproj,
            "bqkv": bqkv,
            "bprojrow": bprojrow,
            "wdwc": wdwc,
            "wdwcn": wdwcn,
            "bdwc": bdwc,
        })

    res = run_bass_kernel_spmd(nc, in_maps, list(range(NCORES)), trace=_trace)
    _cache["last_result"] = res

    out = np.empty((NCORES, NIMG, N, C), dtype=np.float32)
    for c in range(NCORES):
        y5 = res.results[c]["yT"]  # [pair, p, kc, b, t]
        out[c] = y5.transpose(0, 3, 4, 2, 1).reshape(NIMG, N, C)
    return out.reshape(B, N, C)
